# revision 4
# baseline (speedup 1.0000x reference)
"""Trainium2 Bass kernel for nn_HOPEProjection (LayerNorm -> MLP(2048->512,
GELU) -> Linear(512->96) -> tanh+1, split into 3 heads-tensors).

Contract: kernel(**inputs) takes the FULL inputs from setup_inputs() and
returns the FULL output (tuple of three [16384, 32] float32 arrays), running
the compute data-parallel across 8 NeuronCores.

Self-contained: hardcodes all shapes; does not read any sibling files.
"""

import sys

for _p in ("/opt/trn_rl_repo",):
    if _p not in sys.path:
        sys.path.append(_p)

import numpy as np
import ml_dtypes

import concourse.bacc as bacc
import concourse.mybir as mybir
import concourse.tile as tile
from concourse import bass_utils

# ---- problem constants (hardcoded per contract) ----
P = 128              # SBUF partitions
D = 2048             # d_model
H = 512              # hidden
C = 96               # 3 * n_heads
NH = 32              # n_heads
B = 16384            # batch
NCORES = 8
BS = B // NCORES     # rows per core = 2048
RCW = 512            # row-chunk width (matmul moving dim)
NRC = BS // RCW      # 4 row chunks per core
KC = D // P          # 16 contraction chunks
HT = H // P          # 4 hidden tiles
EPS = 1e-5

F32 = mybir.dt.float32
BF16 = mybir.dt.bfloat16
AF = mybir.ActivationFunctionType

_CACHE = {}


def _build_nc():
    nc = bacc.Bacc("TRN2", target_bir_lowering=False, debug=False)

    xT = nc.dram_tensor("xT", [D, BS], F32, kind="ExternalInput").ap()
    w1 = nc.dram_tensor("w1", [D, H], BF16, kind="ExternalInput").ap()
    w2 = nc.dram_tensor("w2", [H, C], BF16, kind="ExternalInput").ap()
    csneg = nc.dram_tensor("csneg", [1, H], BF16, kind="ExternalInput").ap()
    b1c = nc.dram_tensor("b1c", [P, HT], F32, kind="ExternalInput").ap()
    b2r = nc.dram_tensor("b2r", [1, C], BF16, kind="ExternalInput").ap()
    ones_col = nc.dram_tensor("ones_col", [P, 1], BF16, kind="ExternalInput").ap()
    ones_row_b = nc.dram_tensor("ones_row_b", [1, RCW], BF16, kind="ExternalInput").ap()
    ones_row_f = nc.dram_tensor("ones_row_f", [1, P], F32, kind="ExternalInput").ap()
    pT = nc.dram_tensor("pT", [C, BS], F32, kind="ExternalOutput").ap()

    with tile.TileContext(nc) as tc:
        _body(tc, xT, w1, w2, csneg, b1c, b2r, ones_col, ones_row_b, ones_row_f, pT)
    nc.compile()
    return nc


def _body(tc, xT, w1, w2, csneg, b1c, b2r, ones_col, ones_row_b, ones_row_f, pT):
    nc = tc.nc
    import contextlib

    ctx = contextlib.ExitStack()
    with ctx:
        const = ctx.enter_context(tc.tile_pool(name="const", bufs=1))
        xbp = ctx.enter_context(tc.tile_pool(name="xb", bufs=NRC))
        x2p = ctx.enter_context(tc.tile_pool(name="x2", bufs=1))
        trp = ctx.enter_context(tc.tile_pool(name="tr", bufs=1))
        axp = ctx.enter_context(tc.tile_pool(name="ax", bufs=2))
        mup = ctx.enter_context(tc.tile_pool(name="mu", bufs=NRC))
        stp = ctx.enter_context(tc.tile_pool(name="st", bufs=1))
        rqp = ctx.enter_context(tc.tile_pool(name="rq", bufs=NRC))
        rqsp = ctx.enter_context(tc.tile_pool(name="rqs", bufs=2))
        zlp = ctx.enter_context(tc.tile_pool(name="zl", bufs=3))
        hp = ctx.enter_context(tc.tile_pool(name="h", bufs=6))
        pcp = ctx.enter_context(tc.tile_pool(name="pc", bufs=NRC))
        outp = ctx.enter_context(tc.tile_pool(name="out", bufs=2))

        ztp = ctx.enter_context(tc.tile_pool(name="zt", bufs=4, space="PSUM"))
        spp = ctx.enter_context(tc.tile_pool(name="sp", bufs=1, space="PSUM"))
        bcp = ctx.enter_context(tc.tile_pool(name="bc", bufs=1, space="PSUM"))
        mm2p = ctx.enter_context(tc.tile_pool(name="m2", bufs=1, space="PSUM"))

        # ---- weights / constants into SBUF ----
        w1s = const.tile([P, KC, H], BF16, tag="w1s")
        nc.sync.dma_start(w1s[:], w1.rearrange("(k p) h -> p k h", p=P))
        w2s = const.tile([P, HT, C], BF16, tag="w2s")
        nc.sync.dma_start(w2s[:], w2.rearrange("(c p) n -> p c n", p=P))
        csneg_s = const.tile([1, H], BF16, tag="csneg")
        nc.sync.dma_start(csneg_s[:], csneg[:])
        b1c_s = const.tile([P, HT], F32, tag="b1c")
        nc.sync.dma_start(b1c_s[:], b1c[:])
        b2r_s = const.tile([1, C], BF16, tag="b2r")
        nc.sync.dma_start(b2r_s[:], b2r[:])
        ones_col_s = const.tile([P, 1], BF16, tag="ones_col")
        nc.sync.dma_start(ones_col_s[:], ones_col[:])
        ones_row_b_s = const.tile([1, RCW], BF16, tag="ones_row_b")
        nc.sync.dma_start(ones_row_b_s[:], ones_row_b[:])
        ones_row_f_s = const.tile([1, P], F32, tag="ones_row_f")
        nc.sync.dma_start(ones_row_f_s[:], ones_row_f[:])
        eps_s = const.tile([1, 1], F32, tag="eps")
        nc.vector.memset(eps_s[:], EPS)

        xb = [None] * NRC
        mu_b = [None] * NRC
        rsq = [None] * NRC
        pc = [None] * NRC

        # ================= Phase A: loads + LN statistics =================
        for rc in range(NRC):
            xb[rc] = xbp.tile([P, KC, RCW], BF16, tag="xb", name=f"xb{rc}")
            # HBM fp32 -> SBUF bf16 cast during DMA (SWDGE)
            src = xT[:, rc * RCW : (rc + 1) * RCW].rearrange("(k p) r -> p k r", p=P)
            nc.gpsimd.dma_start(xb[rc][:], src)

            # squares on ACT ('square' is a filler fn: no table switch)
            x2 = x2p.tile([P, KC, RCW], BF16, tag="x2")
            for k in range(KC):
                nc.scalar.activation(x2[:, k, :], xb[rc][:, k, :], AF.Square)

            # binary-tree partial sums over the 16 k-chunks (DVE, bf16 2x)
            t8 = trp.tile([P, 8, RCW], BF16, tag="t8")
            nc.vector.tensor_add(t8[:], xb[rc][:, 0:8, :], xb[rc][:, 8:16, :])
            t4 = trp.tile([P, 4, RCW], BF16, tag="t4")
            nc.vector.tensor_add(t4[:], t8[:, 0:4, :], t8[:, 4:8, :])
            t2 = trp.tile([P, 2, RCW], BF16, tag="t2")
            nc.vector.tensor_add(t2[:], t4[:, 0:2, :], t4[:, 2:4, :])
            ax = axp.tile([P, RCW], BF16, tag="ax")
            nc.vector.tensor_add(ax[:], t2[:, 0, :], t2[:, 1, :])

            u8 = trp.tile([P, 8, RCW], BF16, tag="u8")
            nc.vector.tensor_add(u8[:], x2[:, 0:8, :], x2[:, 8:16, :])
            u4 = trp.tile([P, 4, RCW], BF16, tag="u4")
            nc.vector.tensor_add(u4[:], u8[:, 0:4, :], u8[:, 4:8, :])
            u2 = trp.tile([P, 2, RCW], BF16, tag="u2")
            nc.vector.tensor_add(u2[:], u4[:, 0:2, :], u4[:, 2:4, :])
            ax2 = axp.tile([P, RCW], BF16, tag="ax2")
            nc.vector.tensor_add(ax2[:], u2[:, 0, :], u2[:, 1, :])

            # partition reduction via ones-matmul: S1 (p0), S2 (p32)
            sp = spp.tile([33, RCW], F32, tag="sp")
            nc.tensor.matmul(sp[0:1, :], ones_col_s[:], ax[:], start=True, stop=True)
            nc.tensor.matmul(sp[32:33, :], ones_col_s[:], ax2[:], start=True, stop=True)

            # finalize: mu (bf16 row), var, sigma=sqrt(var+eps), rsq=1/sigma
            mu_b[rc] = mup.tile([1, RCW], BF16, tag="mu", name=f"mu{rc}")
            nc.vector.tensor_scalar_mul(mu_b[rc][:], sp[0:1, :], 1.0 / D)
            msq = stp.tile([1, RCW], F32, tag="msq")
            nc.vector.tensor_scalar_mul(msq[:], sp[32:33, :], 1.0 / D)
            mu2 = stp.tile([1, RCW], F32, tag="mu2")
            nc.vector.tensor_mul(mu2[:], mu_b[rc][:], mu_b[rc][:])
            var = stp.tile([1, RCW], F32, tag="var")
            nc.vector.tensor_sub(var[:], msq[:], mu2[:])
            sig = stp.tile([1, RCW], F32, tag="sig")
            nc.scalar.activation(sig[:], var[:], AF.Sqrt, bias=eps_s[:])
            rsq[rc] = rqp.tile([1, RCW], F32, tag="rq", name=f"rq{rc}")
            nc.vector.reciprocal_approx_fast(rsq[rc][:], sig[:])

        # ================= Phase B: MLP =================
        for rc in range(NRC):
            # broadcast rsq row to 128 partitions via K=1 matmul (fp32)
            rqB = bcp.tile([P, RCW], F32, tag="rqB")
            nc.tensor.matmul(rqB[:], ones_row_f_s[:], rsq[rc][:], start=True, stop=True)
            rqS = rqsp.tile([P, RCW], F32, tag="rqS")
            nc.scalar.copy(rqS[:], rqB[:])

            hts = []
            for ht in range(HT):
                zt = ztp.tile([P, RCW], F32, tag="zt")
                for k in range(KC):
                    nc.tensor.matmul(
                        zt[:],
                        w1s[:, k, ht * P : (ht + 1) * P],
                        xb[rc][:, k, :],
                        start=(k == 0),
                        stop=False,
                    )
                # mean correction: += (-colsum) x mu
                nc.tensor.matmul(
                    zt[:],
                    csneg_s[0:1, ht * P : (ht + 1) * P],
                    mu_b[rc][:],
                    start=False,
                    stop=True,
                )
                zl = zlp.tile([P, RCW], F32, tag="zl")
                nc.vector.tensor_mul(zl[:], zt[:], rqS[:])
                h_t = hp.tile([P, RCW], BF16, tag="h")
                nc.scalar.activation(h_t[:], zl[:], AF.Gelu, bias=b1c_s[:, ht : ht + 1])
                hts.append(h_t)

            pp = mm2p.tile([C, RCW], F32, tag="pp")
            for c4 in range(HT):
                nc.tensor.matmul(
                    pp[:], w2s[:, c4, :], hts[c4][:], start=(c4 == 0), stop=False
                )
            nc.tensor.matmul(pp[:], b2r_s[:], ones_row_b_s[:], start=False, stop=True)
            pc[rc] = pcp.tile([C, RCW], F32, tag="pc", name=f"pc{rc}")
            nc.scalar.copy(pc[rc][:], pp[:])

        # ================= Phase C: tanh + 1 + store =================
        for rc in range(NRC):
            th = outp.tile([C, RCW], F32, tag="th")
            nc.scalar.activation(th[:], pc[rc][:], AF.Tanh)
            oo = outp.tile([C, RCW], F32, tag="oo")
            nc.vector.tensor_scalar_add(oo[:], th[:], 1.0)
            nc.sync.dma_start(pT[:, rc * RCW : (rc + 1) * RCW], oo[:])


def _get_nc():
    if "nc" not in _CACHE:
        _CACHE["nc"] = _build_nc()
    return _CACHE["nc"]


def _prep_consts(ln_gamma, ln_beta, W1, b1, W2, b2):
    bf16 = ml_dtypes.bfloat16
    W1p = (W1 * ln_gamma[:, None]).astype(np.float32)
    b1p = (b1 + ln_beta @ W1).astype(np.float32)
    return {
        "w1": np.ascontiguousarray(W1p.astype(bf16)),
        "w2": np.ascontiguousarray(W2.astype(bf16)),
        "csneg": (-W1p.sum(axis=0)).astype(bf16).reshape(1, H),
        "b1c": np.ascontiguousarray(b1p.reshape(HT, P).T.astype(np.float32)),
        "b2r": b2.astype(bf16).reshape(1, C),
        "ones_col": np.ones((P, 1), dtype=bf16),
        "ones_row_b": np.ones((1, RCW), dtype=bf16),
        "ones_row_f": np.ones((1, P), dtype=np.float32),
    }


def _run(nc, in_maps, **kw):
    return bass_utils.run_bass_kernel_spmd(
        nc, in_maps, core_ids=list(range(NCORES)), **kw
    )


def kernel(slow_state, ln_gamma, ln_beta, W1, b1, W2, b2, _bench_kw=None):
    slow_state = np.asarray(slow_state, dtype=np.float32)
    nc = _get_nc()
    consts = _prep_consts(
        np.asarray(ln_gamma, np.float32),
        np.asarray(ln_beta, np.float32),
        np.asarray(W1, np.float32),
        np.asarray(b1, np.float32),
        np.asarray(W2, np.float32),
        np.asarray(b2, np.float32),
    )
    in_maps = []
    for c in range(NCORES):
        shard = slow_state[c * BS : (c + 1) * BS, :]
        m = dict(consts)
        m["xT"] = np.ascontiguousarray(shard.T)
        in_maps.append(m)
    res = _run(nc, in_maps, **(_bench_kw or {}))
    if _bench_kw:
        _CACHE["last_result"] = res
    params = np.concatenate(
        [res.results[c]["pT"].T for c in range(NCORES)], axis=0
    )  # [B, C]
    pr = params.reshape(B, NH, 3)
    return (
        np.ascontiguousarray(pr[..., 0]),
        np.ascontiguousarray(pr[..., 1]),
        np.ascontiguousarray(pr[..., 2]),
    )
